# revision 25
# baseline (speedup 1.0000x reference)
"""DirMagGCNConv (magnetic directed GCN conv) Trainium2 Bass kernel.

out = [ALPHA*lin1 + (1-ALPHA)*lin2](y_re) || same(y_im), where
(y_re, y_im) = magnetic-Laplacian SPMM of x over the symmetrized edge set.

Since q = 0.25, theta in {0, +-pi/2}: reciprocated directed edges contribute
only to the real part (cos=1), unreciprocated ones only to the imaginary
part (sin=+-1). The two linear layers fuse: W = a*W1+(1-a)*W2, b likewise.

Strategy (8 NeuronCores, SPMD single program, destination sharding). The
kernel is DMA-byte-roofline bound; measurements that shaped it:
  - dma_gather descriptor generation runs at best ~2.4ns/idx aggregate
    (4 SWDGE queues, one Q7 cpu pair each) -> 385us/core for 160k edge
    rows, and random 256B-row gather moves bytes at ~half the rate of
    contiguous DMA on the shared SDMA engines. Streaming pre-gathered
    rows is therefore strictly faster: the same bytes at ~2x the DMA
    rate with zero descriptor-generation cost.
  - So the host gathers all edge rows (x[col] in bf16) into per-core,
    per-block contiguous "blobs" that also carry a compact (chunk-band
    index, value) encoding of the one-hot S matrices; the device
    consumes one fat ~1MB DMA per 128-slot destination block,
    alternating between the two HWDGE rings (nc.sync / nc.scalar) to
    hide issue latency, and expands S on the otherwise-idle GPSIMD
    engine with local_scatter (~1.7us per block, 8x fewer S bytes).
  - Destination nodes are bin-packed into 32-slot windows whose in-edge
    counts are close to multiples of 128 (4 windows = one block of 128
    dest slots in PSUM).
  - Per 128-edge chunk one bf16 matmul accumulates into PSUM:
      psum[feat, dest_slots] += G[edges,feat].T @ S[edges, slots]
    then per block one fp32 matmul applies the fused linear layer
    out[slots,:] = yT[feat,slots].T @ W. Outputs are written bf16,
    batched 4 blocks per DMA; bias is added on the host.
  - The ~70 reciprocated-edge copies per core run as one aux chunk into
    a separate 128-slot output; the host adds those y_re@W rows into
    the bias-only real half during unsharding.
"""

import math
import numpy as np
import ml_dtypes

BF16 = ml_dtypes.bfloat16

N_NODES = 40000
N_EDGES = 640000
D = 128
ALPHA = np.float32(0.5)
Q = 0.25
N_CORES = 8
ROWS_PER_CORE = N_NODES // N_CORES  # 5000
WIN_SLOTS = 32          # nodes per window == S width of window chunks
                        # (PSUM matmul out offsets must be 32-float aligned)
WIN_CAP_MAX = 8         # target chunks per window for the bin packing
WINS_PER_BLOCK = 4      # 4 windows * 32 slots = 128 dest slots per block
CHUNK = 128             # edges per chunk == matmul contraction dim
OUT_BATCH = 4           # blocks per output DMA
BLOB_BATCH = 4          # blocks per input blob DMA (~4MB fat transfers)


# ----------------------------------------------------------------- host math
def _edge_values(edge_index):
    """Replicate the reference's symmetrization + magnetic scaling in fp32."""
    row = edge_index[0].astype(np.int64)
    col = edge_index[1].astype(np.int64)
    e = row.shape[0]
    keys = row * N_NODES + col
    sk = np.sort(keys)
    rk = col * N_NODES + row
    pos = np.searchsorted(sk, rk)
    has_rev = (pos < e) & (sk[np.clip(pos, 0, e - 1)] == rk)

    r_all = np.concatenate([row, col])
    c_all = np.concatenate([col, row])
    sign = np.concatenate(
        [np.ones(e, np.float32), -np.ones(e, np.float32)])
    hr = np.concatenate([has_rev, has_rev])
    theta = (np.float32(2.0 * np.pi * Q) * sign
             * (np.float32(1.0) - hr.astype(np.float32)))
    deg = (np.bincount(r_all, minlength=N_NODES).astype(np.float32)
           * np.float32(0.5))
    dinv = np.where(deg > 0, np.float32(1.0) / np.sqrt(deg), np.float32(0.0))
    scale = (np.float32(0.5) * dinv[r_all]) * dinv[c_all]
    val_re = scale * np.cos(theta)
    val_im = scale * np.sin(theta)
    return r_all, c_all, hr, val_re, val_im


def _pack_core(deg_nodes):
    """Bin-pack nodes (by im-degree) into <=WIN_SLOTS-node windows with
    edge capacity WIN_CAP_MAX*CHUNK, minimizing total ceil(degsum/128)."""
    import bisect
    order = np.argsort(-deg_nodes, kind="stable")
    cap = WIN_CAP_MAX * CHUNK
    bins = []            # [nodes, degsum]
    residuals = []       # sorted (residual, bin_id)
    for n in order:
        d = int(deg_nodes[n])
        placed = False
        i = bisect.bisect_left(residuals, (d, -1))
        while i < len(residuals):
            res, bi = residuals[i]
            if len(bins[bi][0]) < WIN_SLOTS:
                residuals.pop(i)
                bins[bi][0].append(int(n))
                bins[bi][1] += d
                bisect.insort(residuals, (cap - bins[bi][1], bi))
                placed = True
                break
            i += 1
        if not placed:
            bins.append([[int(n)], d])
            bisect.insort(residuals, (cap - d, len(bins) - 1))
    return bins


def _preprocess(x, edge_index):
    """Build per-core device blobs + the shared program-shape metadata."""
    r_all, c_all, hr, val_re, val_im = _edge_values(edge_index)
    im = ~hr
    core_of = r_all // ROWS_PER_CORE
    deg_im = np.bincount(r_all[im], minlength=N_NODES)

    # ---- pack each core; shared window-capacity profile
    core_bins, core_needs = [], []
    for c in range(N_CORES):
        nodes = slice(c * ROWS_PER_CORE, (c + 1) * ROWS_PER_CORE)
        bins = _pack_core(deg_im[nodes])
        needs = sorted((max(1, math.ceil(b[1] / CHUNK)) for b in bins),
                       reverse=True)
        core_bins.append(bins)
        core_needs.append(needs)
    nw = max(len(n) for n in core_needs)
    nw = ((nw + WINS_PER_BLOCK - 1) // WINS_PER_BLOCK) * WINS_PER_BLOCK
    profile = np.zeros(nw, np.int64)
    for needs in core_needs:
        profile[: len(needs)] = np.maximum(profile[: len(needs)], needs)
    nblk = nw // WINS_PER_BLOCK

    perm_slot = np.full((N_CORES, ROWS_PER_CORE), -1, np.int64)
    for c in range(N_CORES):
        bins = core_bins[c]
        order = sorted(range(len(bins)),
                       key=lambda i: -max(1, math.ceil(bins[i][1] / CHUNK)))
        for w, bi in enumerate(order):
            for s, n in enumerate(bins[bi][0]):
                perm_slot[c, n] = w * WIN_SLOTS + s
    assert (perm_slot >= 0).all()

    dest_local = r_all % ROWS_PER_CORE
    e_slot = perm_slot[core_of, dest_local]
    e_win = e_slot // WIN_SLOTS
    KL = [int(profile[b * WINS_PER_BLOCK:(b + 1) * WINS_PER_BLOCK].sum())
          for b in range(nblk)]

    # aux (reciprocated) edges: one chunk for the whole core
    for c in range(N_CORES):
        assert (core_of == c)[hr].sum() <= CHUNK, "re chunk overflow"

    # blob layout: per block [KL*CHUNK gathered-row cols || KLe int16 idx
    # cols || KLe bf16 val cols] where KLe = KL rounded up to even (S is
    # expanded on-device by local_scatter; idx = chunk*32 + slot%32, -1
    # pads ignored). aux blob [CHUNK row cols || 2 idx || 2 val] at the end.
    KLe = [KL[b] + (KL[b] & 1) for b in range(nblk)]
    blk_off = []
    off = 0
    for b in range(nblk):
        blk_off.append(off)
        off += KL[b] * CHUNK + 2 * KLe[b]
    aux_off = off
    blob_cols = off + CHUNK + 4

    per_core = []
    val_eff = np.where(hr, val_re, val_im).astype(np.float32)
    xbf = x.astype(BF16)
    aux_maps = []
    for c in range(N_CORES):
        blob = np.zeros((128, blob_cols), BF16)

        mc = core_of == c
        ew, es = e_win[mc], e_slot[mc]
        src, vv = c_all[mc], val_eff[mc]
        e_hr = hr[mc]

        for b in range(nblk):
            if KL[b] == 0:
                continue
            gb = np.zeros((128, KL[b], 128), BF16)
            six = np.full((128, KLe[b]), -1, np.int16)
            sva = np.zeros((128, KLe[b]), np.float32)
            lc = 0
            for gw in range(b * WINS_PER_BLOCK, (b + 1) * WINS_PER_BLOCK):
                cap = int(profile[gw])
                sel = np.nonzero((ew == gw) & ~e_hr)[0]
                assert len(sel) <= cap * CHUNK
                j = np.arange(len(sel))
                gb[j % CHUNK, lc + j // CHUNK, :] = xbf[src[sel]]
                scol = (es[sel] % WIN_SLOTS).astype(np.int64)
                six[j % CHUNK, lc + j // CHUNK] = \
                    ((lc + j // CHUNK) * WIN_SLOTS + scol).astype(np.int16)
                sva[j % CHUNK, lc + j // CHUNK] = vv[sel]
                lc += cap
            assert lc == KL[b]
            o = blk_off[b]
            blob[:, o: o + KL[b] * CHUNK] = gb.reshape(128, -1)
            blob[:, o + KL[b] * CHUNK: o + KL[b] * CHUNK + KLe[b]] = \
                six.view(BF16)
            blob[:, o + KL[b] * CHUNK + KLe[b]:
                 o + KL[b] * CHUNK + 2 * KLe[b]] = sva.astype(BF16)

        # aux re chunk; aux slot = per-core re-dest index
        re_idx = np.nonzero(e_hr)[0]
        re_dests = np.unique(es[re_idx])
        slot_of = {int(s): i for i, s in enumerate(re_dests)}
        assert len(re_dests) <= 128
        j = np.arange(len(re_idx))
        ga = np.zeros((128, 128), BF16)
        aix = np.full((128, 2), -1, np.int16)
        ava = np.zeros((128, 2), np.float32)
        ga[j, :] = xbf[src[re_idx]]
        aix[j, 0] = np.array([slot_of[int(s)] for s in es[re_idx]], np.int16)
        ava[j, 0] = vv[re_idx]
        blob[:, aux_off: aux_off + CHUNK] = ga
        blob[:, aux_off + CHUNK: aux_off + CHUNK + 2] = aix.view(BF16)
        blob[:, aux_off + CHUNK + 2: aux_off + CHUNK + 4] = \
            ava.astype(BF16)

        # node ids (global) for each aux slot, for the host-side merge
        core_nodes = np.arange(c * ROWS_PER_CORE, (c + 1) * ROWS_PER_CORE)
        pslot = perm_slot[c]
        inv = np.full(nw * WIN_SLOTS, -1, np.int64)
        inv[pslot] = core_nodes
        aux_nodes = inv[re_dests]
        assert (aux_nodes >= 0).all()
        aux_maps.append(aux_nodes)

        per_core.append(dict(blob=blob))

    meta = dict(profile=profile, KL=KL, KLe=KLe, nblk=nblk,
                blk_off=blk_off, aux_off=aux_off, blob_cols=blob_cols,
                perm_slot=perm_slot, aux_maps=aux_maps)
    return meta, per_core


# ------------------------------------------------------------ device program
def _build_program(meta):
    import concourse.bacc as bacc
    import concourse.tile as tile
    import concourse.mybir as mybir

    fp32 = mybir.dt.float32
    bf16 = mybir.dt.bfloat16
    i16 = mybir.dt.int16
    nblk = meta["nblk"]
    KL = meta["KL"]
    KLe = meta["KLe"]
    profile = meta["profile"]
    blk_off = meta["blk_off"]
    aux_off = meta["aux_off"]
    blob_cols = meta["blob_cols"]
    n_groups = (nblk + OUT_BATCH - 1) // OUT_BATCH

    nc = bacc.Bacc("TRN2", target_bir_lowering=False)
    blob_d = nc.dram_tensor("blob", [128, blob_cols], bf16,
                            kind="ExternalInput")
    wmat_d = nc.dram_tensor("wmat", [128, 128], fp32, kind="ExternalInput")
    czero_d = nc.dram_tensor("czero", [1, 128], bf16, kind="ExternalInput")
    out_d = nc.dram_tensor("out", [128, n_groups * OUT_BATCH * 128], bf16,
                           kind="ExternalOutput")
    outaux_d = nc.dram_tensor("outaux", [128, 128], fp32,
                              kind="ExternalOutput")

    with tile.TileContext(nc) as tc:
        with (
            tc.tile_pool(name="const", bufs=1) as cpool,
            tc.tile_pool(name="blob", bufs=4) as blob_pool,
            tc.tile_pool(name="sv", bufs=6) as sv_pool,
            tc.tile_pool(name="yt", bufs=4) as y_pool,
            tc.tile_pool(name="obat", bufs=2) as o_pool,
            tc.tile_pool(name="oaux", bufs=1) as oa_pool,
            tc.tile_pool(name="ps", bufs=4, space="PSUM") as ps_pool,
            tc.tile_pool(name="pso", bufs=4, space="PSUM") as pso_pool,
        ):
            wmat_t = cpool.tile([128, 128], fp32)
            nc.sync.dma_start(wmat_t[:], wmat_d[:])
            czero_t = cpool.tile([1, 128], bf16)
            nc.sync.dma_start(czero_t[:], czero_d[:])

            obat = None
            gblob = None
            goff = 0
            for b in range(nblk):
                if b % OUT_BATCH == 0:
                    obat = o_pool.tile([128, OUT_BATCH * 128], bf16,
                                       tag="ob")
                if b % BLOB_BATCH == 0:
                    b1 = min(b + BLOB_BATCH, nblk)
                    gend = aux_off if b1 == nblk else blk_off[b1]
                    goff = blk_off[b]
                    gcols = gend - goff
                    if gcols > 0:
                        gblob = blob_pool.tile([128, gcols], bf16,
                                               tag="blob")
                        eng = (nc.sync if (b // BLOB_BATCH) % 2 == 0
                               else nc.scalar)
                        eng.dma_start(gblob[:],
                                      blob_d[:, goff: goff + gcols])
                if KL[b] > 0:
                    o = blk_off[b] - goff
                    blob = gblob[:, o: o + KL[b] * CHUNK + 2 * KLe[b]]
                    sv = sv_pool.tile([128, KL[b] * WIN_SLOTS], bf16,
                                      tag="sv")
                    nc.gpsimd.local_scatter(
                        sv[:],
                        blob[:, KL[b] * CHUNK + KLe[b]:
                             KL[b] * CHUNK + 2 * KLe[b]],
                        blob[:, KL[b] * CHUNK:
                             KL[b] * CHUNK + KLe[b]].bitcast(i16),
                        channels=128, num_elems=KL[b] * WIN_SLOTS,
                        num_idxs=KLe[b])

                    ps = ps_pool.tile([128, 128], fp32, tag="ps")
                    # K=1 zero matmul clears the whole bank so start flags
                    # stay uniform (windows can have 0 chunks for a core).
                    nc.tensor.matmul(ps[:, :], czero_t[:], czero_t[:],
                                     start=True, stop=False)
                    lc = 0
                    for gw in range(b * WINS_PER_BLOCK,
                                    (b + 1) * WINS_PER_BLOCK):
                        cap = int(profile[gw])
                        if cap == 0:
                            continue
                        col0 = (gw % WINS_PER_BLOCK) * WIN_SLOTS
                        for k in range(cap):
                            nc.tensor.matmul(
                                ps[:, col0: col0 + WIN_SLOTS],
                                blob[:, (lc + k) * CHUNK:
                                     (lc + k + 1) * CHUNK],
                                sv[:, (lc + k) * WIN_SLOTS:
                                   (lc + k + 1) * WIN_SLOTS],
                                start=False,
                                stop=(gw == (b + 1) * WINS_PER_BLOCK - 1
                                      and k == cap - 1))
                        lc += cap
                    assert lc == KL[b]

                    ytb = y_pool.tile([128, 128], fp32, tag="yt")
                    nc.vector.tensor_copy(ytb[:], ps[:])
                    pso = pso_pool.tile([128, 128], fp32, tag="pso")
                    nc.tensor.matmul(pso[:, :], ytb[:, :], wmat_t[:],
                                     start=True, stop=True)
                    nc.vector.tensor_copy(
                        obat[:, (b % OUT_BATCH) * 128:
                             (b % OUT_BATCH + 1) * 128], pso[:])
                if b % OUT_BATCH == OUT_BATCH - 1 or b == nblk - 1:
                    g = b // OUT_BATCH
                    nc.scalar.dma_start(
                        out_d[:, g * OUT_BATCH * 128:
                              (g + 1) * OUT_BATCH * 128], obat[:])

            # ---- aux pass: reciprocated edges -> y_re @ W rows
            ba = blob_pool.tile([128, CHUNK + 4], bf16, tag="blob")
            nc.sync.dma_start(ba[:], blob_d[:, aux_off: aux_off + CHUNK + 4])
            sa = sv_pool.tile([128, 128], bf16, tag="sv")
            nc.gpsimd.local_scatter(
                sa[:], ba[:, CHUNK + 2: CHUNK + 4],
                ba[:, CHUNK: CHUNK + 2].bitcast(i16),
                channels=128, num_elems=128, num_idxs=2)
            pa = ps_pool.tile([128, 128], fp32, tag="ps")
            nc.tensor.matmul(pa[:, :], czero_t[:], czero_t[:],
                             start=True, stop=False)
            nc.tensor.matmul(pa[:, :], ba[:, 0:CHUNK], sa[:],
                             start=False, stop=True)
            yta = y_pool.tile([128, 128], fp32, tag="yt")
            nc.vector.tensor_copy(yta[:], pa[:])
            poa = pso_pool.tile([128, 128], fp32, tag="pso")
            nc.tensor.matmul(poa[:, :], yta[:, :], wmat_t[:],
                             start=True, stop=True)
            oba = oa_pool.tile([128, 128], fp32)
            nc.vector.tensor_copy(oba[:], poa[:])
            nc.sync.dma_start(outaux_d[:, :], oba[:])

    nc.compile()
    return nc


def kernel(x, edge_index, W1, b1, W2, b2):
    x = np.asarray(x, dtype=np.float32)
    edge_index = np.asarray(edge_index)
    W1 = np.asarray(W1, dtype=np.float32)
    b1 = np.asarray(b1, dtype=np.float32)
    W2 = np.asarray(W2, dtype=np.float32)
    b2 = np.asarray(b2, dtype=np.float32)

    from concourse.bass_utils import run_bass_kernel_spmd

    meta, per_core = _preprocess(x, edge_index)
    nc = _build_program(meta)

    wmat = (ALPHA * W1 + (np.float32(1.0) - ALPHA) * W2).astype(np.float32)
    brow = (ALPHA * b1 + (np.float32(1.0) - ALPHA) * b2).astype(np.float32)

    in_maps = []
    for c in range(N_CORES):
        in_maps.append({
            "blob": per_core[c]["blob"],
            "wmat": wmat,
            "czero": np.zeros((1, 128), BF16),
        })

    res = run_bass_kernel_spmd(nc, in_maps, core_ids=list(range(N_CORES)))

    nblk = meta["nblk"]
    out = np.empty((N_NODES, 2 * D), np.float32)
    out[:, 0:D] = brow
    perm_slot = meta["perm_slot"]
    for c in range(N_CORES):
        dev = res.results[c]["out"].astype(np.float32)  # [128, NG*4*128]
        nb_pad = dev.shape[1] // 128
        rows = dev.reshape(128, nb_pad, 128).transpose(1, 0, 2) \
                  .reshape(nb_pad * 128, 128)
        out[c * ROWS_PER_CORE:(c + 1) * ROWS_PER_CORE, D:2 * D] = \
            rows[perm_slot[c]] + brow
        aux_nodes = meta["aux_maps"][c]
        if len(aux_nodes):
            out[aux_nodes, 0:D] += res.results[c]["outaux"][: len(aux_nodes)]
    return out
